# revision 12
# baseline (speedup 1.0000x reference)
"""Trainium2 Bass kernel for 16-head causal MHA (B=4, S=2048, D=1024).

Sharding: 8 cores = 4 batches x 2 head-groups (8 heads each).
Per core (batch b, head-group hg):
  inputs:  XT = X[b].T [1024,2048], WQ/WK/WV column shards [1024,512],
           WO row shard [512,1024], bias shards, causal mask tiles.
  output:  YT = (A_hg @ WO_hg + bo*[hg==0]).T  [1024, 2048]  (partial)
Host combine: Y[b] = (YT[2b] + YT[2b+1]).T

On-core dataflow (everything "transposed" so no on-device transposes):
  Q^T,K^T [512,2048] and V [2048,512] via fp32r matmuls;
  scores^T[sk,sq] = K_h @ Q_h^T; exp on ACT (scale=1/8 folded);
  causal mask multiplied on diagonal chunks;
  O^T_aug = [V_h | 1]^T @ attn^T  -> row 64 = softmax denominators;
  normalizer broadcast via K=1 matmul with ones; A^T scaled in place;
  Y^T = WO^T @ A^T (+bo as per-partition ACT bias).
"""

import sys

import numpy as np

_REPO = "/opt/trn_rl_repo"

B, S, D = 4, 2048, 1024
H, DK = 16, 64
HPC = 8            # heads per core
MD = HPC * DK      # 512: per-core head width
P = 128
SB = 512           # s-block
NSB = S // SB      # 4
NDC = D // P       # 8
NMC = MD // P      # 4
NSC = S // P       # 16
GRP = 2            # sk-chunks per exp group

_CACHE = {}


def _ensure_path():
    try:
        import concourse  # noqa: F401
    except ImportError:
        if _REPO not in sys.path:
            sys.path.insert(0, _REPO)


def _build():
    _ensure_path()
    from contextlib import ExitStack

    import concourse.bass as bass  # noqa: F401
    import concourse.mybir as mybir
    import concourse.tile as tile
    from concourse import bacc

    dt = mybir.dt
    f32 = dt.float32
    f32r = dt.float32r
    AF = mybir.ActivationFunctionType

    nc = bacc.Bacc(None, target_bir_lowering=False)
    XT = nc.dram_tensor("XT", [D, S], f32r, kind="ExternalInput")
    WQ = nc.dram_tensor("WQ", [D, MD], f32r, kind="ExternalInput")
    WK = nc.dram_tensor("WK", [D, MD], f32r, kind="ExternalInput")
    WV = nc.dram_tensor("WV", [D, MD], f32r, kind="ExternalInput")
    WO = nc.dram_tensor("WO", [MD, D], f32r, kind="ExternalInput")
    BQ = nc.dram_tensor("BQ", [MD], f32, kind="ExternalInput")
    BK = nc.dram_tensor("BK", [MD], f32, kind="ExternalInput")
    BV = nc.dram_tensor("BV", [MD], f32r, kind="ExternalInput")
    BO = nc.dram_tensor("BO", [D], f32, kind="ExternalInput")
    MASKS = nc.dram_tensor("MASKS", [4, P, SB], f32r, kind="ExternalInput")
    YT = nc.dram_tensor("YT", [D, S], f32, kind="ExternalOutput")

    with ExitStack() as ctx:
        ctx.enter_context(nc.allow_low_precision(reason="fp32r matmul pipeline"))
        tc = ctx.enter_context(tile.TileContext(nc))
        consts = ctx.enter_context(tc.tile_pool(name="consts", bufs=1))
        qkv = ctx.enter_context(tc.tile_pool(name="qkv", bufs=1))
        atp = ctx.enter_context(tc.tile_pool(name="atp", bufs=1))

        # Dummy first ACT op: walrus attaches the ACT table-load pseudo to the
        # first activation; keep its sync-wait list minimal.
        dummy = consts.tile([1, 16], f32)
        nc.vector.memset(dummy[:], 0.0)
        nc.scalar.activation(dummy[:], dummy[:], AF.Exp)
        nc.scalar.activation(dummy[:], dummy[:], AF.Identity)

        masks_t = consts.tile([P, 4, SB], f32r)
        nc.sync.dma_start(masks_t[:], MASKS.rearrange("i p n -> p i n"))
        ones_t = consts.tile([1, P], f32r)
        nc.vector.memset(ones_t[:].bitcast(f32), 1.0)
        bqt = consts.tile([P, NMC], f32)
        nc.sync.dma_start(bqt[:], BQ.rearrange("(c p) -> p c", p=P))
        bkt = consts.tile([P, NMC], f32)
        nc.sync.dma_start(bkt[:], BK.rearrange("(c p) -> p c", p=P))
        bvt = consts.tile([1, MD], f32r)
        nc.sync.dma_start(bvt[:], BV[None, :])
        bot = consts.tile([P, NDC], f32)
        nc.sync.dma_start(bot[:], BO.rearrange("(c p) -> p c", p=P))

        kt = qkv.tile([P, NMC, S], f32r)            # K^T  (m-chunk, sk)
        vaug = qkv.tile([P, NSC, HPC, DK + 1], f32r)  # V per s-chunk + ones col
        at = atp.tile([P, NMC, S], f32r)            # A^T accumulates heads

        with (
            tc.tile_pool(name="xt", bufs=1) as xtp,
            tc.tile_pool(name="wst", bufs=2) as wst,
            tc.tile_pool(name="qt", bufs=2) as qtp,
            tc.tile_pool(name="attn", bufs=3) as attnp,
            tc.tile_pool(name="rs", bufs=2) as rsp,
            tc.tile_pool(name="pp", bufs=2, space="PSUM") as pp,
            tc.tile_pool(name="psc", bufs=2, space="PSUM") as psc,
            tc.tile_pool(name="po", bufs=1, space="PSUM") as pop,
            tc.tile_pool(name="pn", bufs=1, space="PSUM") as pnp,
        ):
            for sb in range(NSB):
                # ---------- projections for s-block sb ----------
                xt_t = xtp.tile([P, NDC, SB], f32r)
                for dc in range(NDC):
                    nc.sync.dma_start(
                        xt_t[:, dc, :], XT[dc * P:(dc + 1) * P, sb * SB:(sb + 1) * SB]
                    )
                qt_t = qtp.tile([P, NMC, SB], f32r)

                wq_t = wst.tile([P, NDC, MD], f32r, tag="w")
                nc.sync.dma_start(wq_t[:], WQ.rearrange("(c p) m -> p c m", p=P))
                for mc in range(NMC):
                    ps = pp.tile([P, SB], f32)
                    for dc in range(NDC):
                        nc.tensor.matmul(
                            ps[:],
                            (wq_t[:, dc, mc * P:(mc + 1) * P]),
                            (xt_t[:, dc, :]),
                            start=(dc == 0),
                            stop=(dc == NDC - 1),
                        )
                    nc.scalar.activation(
                        qt_t[:, mc, :], ps[:], AF.Identity, bias=bqt[:, mc:mc + 1]
                    )

                wk_t = wst.tile([P, NDC, MD], f32r, tag="w")
                nc.sync.dma_start(wk_t[:], WK.rearrange("(c p) m -> p c m", p=P))
                for mc in range(NMC):
                    ps = pp.tile([P, SB], f32)
                    for dc in range(NDC):
                        nc.tensor.matmul(
                            ps[:],
                            (wk_t[:, dc, mc * P:(mc + 1) * P]),
                            (xt_t[:, dc, :]),
                            start=(dc == 0),
                            stop=(dc == NDC - 1),
                        )
                    nc.scalar.activation(
                        kt[:, mc, sb * SB:(sb + 1) * SB], ps[:], AF.Identity,
                        bias=bkt[:, mc:mc + 1],
                    )

                wv_t = wst.tile([P, NDC, MD], f32r, tag="w")
                nc.sync.dma_start(wv_t[:], WV.rearrange("(c p) m -> p c m", p=P))
                for sc in range(SB // P):
                    gsc = sb * (SB // P) + sc
                    ps = pp.tile([P, SB], f32)
                    for dc in range(NDC):
                        nc.tensor.matmul(
                            ps[:],
                            (xt_t[:, dc, sc * P:(sc + 1) * P]),
                            (wv_t[:, dc, :]),
                            start=(dc == 0),
                            stop=False,
                        )
                    nc.tensor.matmul(
                        ps[:], (ones_t[:, :P]), (bvt[:]), start=False, stop=True
                    )
                    nc.vector.tensor_copy(
                        vaug[:, gsc, :, 0:DK],
                        ps.rearrange("p (h d) -> p h d", h=HPC),
                    )
                    nc.gpsimd.memset(vaug[:, gsc, :, DK:DK + 1].bitcast(f32), 1.0)

                # ---------- attention for qsb = sb ----------
                qsb = sb
                nchunks = 4 * qsb + 4
                for h in range(HPC):
                    hc, off = h // 2, (h % 2) * DK
                    po = pop.tile([DK + 1, SB], f32)
                    first = True
                    for g0 in range(0, nchunks, GRP):
                        gn = min(GRP, nchunks - g0)
                        sp = psc.tile([P, GRP * SB], f32)
                        for ci in range(gn):
                            c = g0 + ci
                            nc.tensor.matmul(
                                sp[:, ci * SB:(ci + 1) * SB],
                                (kt[off:off + DK, hc, c * P:(c + 1) * P]),
                                (qt_t[off:off + DK, hc, :]),
                                start=True,
                                stop=True,
                            )
                        at_g = attnp.tile([P, GRP * SB], f32r)
                        nc.scalar.activation(
                            at_g[:, :gn * SB], sp[:, :gn * SB], AF.Exp, scale=0.125
                        )
                        for ci in range(gn):
                            i = g0 + ci - 4 * qsb
                            if i >= 0:
                                nc.vector.tensor_mul(
                                    at_g[:, ci * SB:(ci + 1) * SB],
                                    at_g[:, ci * SB:(ci + 1) * SB],
                                    masks_t[:, i, :],
                                )
                        for ci in range(gn):
                            c = g0 + ci
                            nc.tensor.matmul(
                                po[:],
                                (vaug[:, c, h, :]),
                                (at_g[:, ci * SB:(ci + 1) * SB]),
                                start=first,
                                stop=(c == nchunks - 1),
                            )
                            first = False
                    rs = rsp.tile([1, SB], f32r)
                    nc.vector.reciprocal(rs[:], po[DK:DK + 1, :])
                    pn = pnp.tile([DK, SB], f32)
                    nc.tensor.matmul(
                        pn[:], (ones_t[:, :DK]), (rs[:]), start=True, stop=True
                    )
                    at_sl = at[off:off + DK, hc, qsb * SB:(qsb + 1) * SB]
                    nc.vector.tensor_copy(at_sl, po[0:DK, :])
                    nc.vector.tensor_mul(at_sl, at_sl, pn[:])

        # ---------- output projection Y^T = WO^T @ A^T ----------
        with (
            tc.tile_pool(name="wo", bufs=1) as wop,
            tc.tile_pool(name="yb", bufs=3) as ybp,
            tc.tile_pool(name="py", bufs=4, space="PSUM") as pyp,
        ):
            wo_t = wop.tile([P, NMC, D], f32r)
            nc.sync.dma_start(wo_t[:], WO.rearrange("(c p) d -> p c d", p=P))
            for sb in range(NSB):
                for dc in range(NDC):
                    ps = pyp.tile([P, SB], f32)
                    for hcc in range(NMC):
                        nc.tensor.matmul(
                            ps[:],
                            (wo_t[:, hcc, dc * P:(dc + 1) * P]),
                            (at[:, hcc, sb * SB:(sb + 1) * SB]),
                            start=(hcc == 0),
                            stop=(hcc == NMC - 1),
                        )
                    yb = ybp.tile([P, SB], f32)
                    nc.scalar.activation(
                        yb[:], ps[:], AF.Identity, bias=bot[:, dc:dc + 1]
                    )
                    nc.sync.dma_start(
                        YT[dc * P:(dc + 1) * P, sb * SB:(sb + 1) * SB], yb[:]
                    )
    nc.finalize()
    return nc


def _masks():
    i = np.arange(4)[:, None, None]
    p = np.arange(P)[None, :, None]
    j = np.arange(SB)[None, None, :]
    return (128 * i + p <= j).astype(np.float32)


def _in_maps(X, Wq, bq, Wk, bk, Wv, bv, Wo, bo):
    masks = _masks()
    zeros_bo = np.zeros_like(bo)
    maps = []
    for core in range(8):
        b, hg = core // 2, core % 2
        sl = slice(hg * MD, (hg + 1) * MD)
        maps.append({
            "XT": np.ascontiguousarray(X[b].T),
            "WQ": np.ascontiguousarray(Wq[:, sl]),
            "WK": np.ascontiguousarray(Wk[:, sl]),
            "WV": np.ascontiguousarray(Wv[:, sl]),
            "WO": np.ascontiguousarray(Wo[sl, :]),
            "BQ": np.ascontiguousarray(bq[sl]),
            "BK": np.ascontiguousarray(bk[sl]),
            "BV": np.ascontiguousarray(bv[sl]),
            "BO": bo if hg == 0 else zeros_bo,
            "MASKS": masks,
        })
    return maps


_LAST_RESULTS = None


def kernel(X, Wq, bq, Wk, bk, Wv, bv, Wo, bo):
    global _LAST_RESULTS
    _ensure_path()
    from concourse import bass_utils

    args = [np.ascontiguousarray(np.asarray(a, dtype=np.float32))
            for a in (X, Wq, bq, Wk, bk, Wv, bv, Wo, bo)]
    if "nc" not in _CACHE:
        _CACHE["nc"] = _build()
    nc = _CACHE["nc"]
    res = bass_utils.run_bass_kernel_spmd(nc, _in_maps(*args), core_ids=list(range(8)))
    _LAST_RESULTS = res
    out = np.empty((B, S, D), dtype=np.float32)
    for b in range(B):
        out[b] = (res.results[2 * b]["YT"] + res.results[2 * b + 1]["YT"]).T
    return out
